# revision 13
# baseline (speedup 1.0000x reference)
"""TRN2 Bass kernel for nn_GQA_22436909154699 — optimized v5.

Reference math: softmax over a size-1 axis is identically 1.0, so
    out[b,l,g,h,:] = v[b,l,g,:]          (v = v-half of x @ Wkv + bkv)
The q projection (x @ Wq) never affects the output.  The kernel computes
    y = x @ Wv + bv                      (K=2048, N=256)
data-parallel over tokens across 8 NeuronCores (2048 tokens each).

Measured fixed framework cost is ~10us inside the counted window (~1us
preamble + ~8.6us end-of-NEFF semaphore sweep), so the optimizable span
is [first DMA .. last output landed]:
  - x streams as 16 x 512KB chunks on the Sync HWDGE ring (512KB keeps
    the SDMA engines near line rate; 256KB chunks measured only 78%).
  - weights (k-halves interleaved wv0a,wv1a,wv0b,wv1b) + bias ride the
    Scalar/ACT ring in parallel, so the first matmul only gates on
    ~0.5MB of weights + one x chunk.
  - per-chunk nh-interleave: each 4-ktile chunk feeds both column-half
    PSUM groups immediately; the PE trails the stream by one chunk and
    the post-stream tail is ~1.7us.
  - bf16 outputs, block outputs split across both rings (nh0 on Sync,
    nh1 on ACT) so the final adds + stores overlap.
  - 5 PE warm-ups on zeros cover the HAM un-throttle before real MMs.
"""

import numpy as np

# Problem constants (hardcoded; harness runs kernel.py standalone).
B, L, E = 4, 4096, 2048
G, HPG, D = 4, 8, 64
NV = G * D  # 256 v-columns
NH = NV // 128  # 2 column halves (PE stationary is 128 wide)
NCORES = 8
TOK = B * L  # 16384 tokens
TPC = TOK // NCORES  # 2048 tokens per core
TBLK = 512  # tokens per matmul rhs / PSUM group
TB = TPC // TBLK  # 4 token blocks per core
KO = E // 128  # 16 contraction tiles
XCH = 4  # x chunks per block (512 KB each)
KPC = KO // XCH  # k-tiles per chunk = 4
WH = 2  # weight k-halves per nh (256 KB each)

_CACHE: dict = {}
LAST_RESULTS = None


def _build(warmup: int):
    import concourse.bacc as bacc
    import concourse.mybir as mybir
    import concourse.tile as tile

    F32 = mybir.dt.float32
    BF16 = mybir.dt.bfloat16

    nc = bacc.Bacc(
        "TRN2", target_bir_lowering=False, debug=False, num_devices=NCORES
    )
    xt_d = nc.dram_tensor(
        "xt", [TB, XCH, 128, KPC, TBLK], BF16, kind="ExternalInput"
    )
    wv_d = nc.dram_tensor("wv", [NH, 128, KO, 128], BF16, kind="ExternalInput")
    bias_d = nc.dram_tensor("bias", [128, NH], F32, kind="ExternalInput")
    out_d = nc.dram_tensor("out", [NH, TB, 128, TBLK], BF16, kind="ExternalOutput")

    with tile.TileContext(nc) as tc:
        with (
            tc.tile_pool(name="const", bufs=1) as cpool,
            tc.tile_pool(name="xin", bufs=TB) as xpool,
            tc.tile_pool(name="obuf", bufs=4) as opool,
            tc.tile_pool(name="ps", bufs=8, space="PSUM") as ppool,
        ):
            # PE warm-up on zeros while the first DMAs land.  ~8 N=512 MMs
            # span the ~3.4us HAM busy-window, so the real MM stream starts
            # at K=8/8 (2.4 GHz) with no cold prefix.  GpSimd memset runs at
            # window start (DVE would add ~0.5us of latency).
            if warmup:
                zt = cpool.tile([128, TBLK], BF16)
                nc.gpsimd.memset(zt[:], 0.0)
                wps = ppool.tile([128, TBLK], F32, tag="ps")
                for _ in range(warmup):
                    nc.tensor.matmul(
                        wps[:], lhsT=zt[:, :128], rhs=zt[:], start=True, stop=True
                    )

            # Weights + bias on the ACT HWDGE ring.  One DMA per wv half:
            # Tile has only 8 global DMAHW completion lanes, and extra weight
            # DMAs here starve the x-chunk dispatches of lanes (measured: the
            # 4th x chunk's dispatch stalled ~3us behind weight completions).
            wvs = []
            for nh in range(NH):
                wvs.append(
                    cpool.tile(
                        [128, KO, 128], BF16, tag=f"wv{nh}", name=f"wv{nh}"
                    )
                )
                nc.scalar.dma_start(wvs[nh][:], wv_d[nh])
            bias_sb = cpool.tile([128, NH], F32)
            nc.scalar.dma_start(bias_sb[:], bias_d[:])

            # x stream: 16 x 512KB chunks, FIFO on the Sync HWDGE ring.
            xin = []
            for tb in range(TB):
                xt = xpool.tile([128, KO, TBLK], BF16, tag="xin")
                for c in range(XCH):
                    nc.sync.dma_start(
                        xt[:, c * KPC : (c + 1) * KPC, :], xt_d[tb, c]
                    )
                xin.append(xt)

            for tb in range(TB):
                pss = [
                    ppool.tile([128, TBLK], F32, tag="ps", name=f"ps{tb}_{i}")
                    for i in range(NH)
                ]
                for c in range(XCH):
                    for nh in range(NH):
                        for kk in range(KPC):
                            k = c * KPC + kk
                            nc.tensor.matmul(
                                pss[nh][:],
                                lhsT=wvs[nh][:, k, :],
                                rhs=xin[tb][:, k, :],
                                start=(k == 0),
                                stop=(k == KO - 1),
                            )
                for nh in range(NH):
                    ot = opool.tile([128, TBLK], BF16, tag="ot", name=f"ot{tb}_{nh}")
                    if nh == 0:
                        # DVE add + store on the Sync ring.
                        nc.vector.tensor_add(
                            ot[:],
                            pss[nh][:],
                            bias_sb[:, nh, None].to_broadcast([128, TBLK]),
                        )
                        nc.sync.dma_start(out_d[nh, tb], ot[:])
                    else:
                        # ACT copy-with-bias (faster than the DVE add) + store
                        # on the ACT ring — the nh1 drain is the critical tail.
                        nc.scalar.activation(
                            ot[:],
                            pss[nh][:],
                            mybir.ActivationFunctionType.Identity,
                            bias=bias_sb[:, nh, None],
                        )
                        nc.scalar.dma_start(out_d[nh, tb], ot[:])
    nc.compile()
    return nc


def _get_nc():
    warmup = 8
    key = ("nc7", warmup)
    if key not in _CACHE:
        _CACHE[key] = _build(warmup)
    return _CACHE[key]


def _to_bf16(a):
    import ml_dtypes

    return a.astype(ml_dtypes.bfloat16)


def _prep_inputs(x, Wkv, bkv):
    x = np.asarray(x, dtype=np.float32).reshape(TOK, E)
    Wkv = np.asarray(Wkv, dtype=np.float32)
    bkv = np.asarray(bkv, dtype=np.float32)

    xb = _to_bf16(x)
    # (core, tb, t, c, kk, p) -> (core, tb, c, p, kk, t)
    xt = xb.reshape(NCORES, TB, TBLK, XCH, KPC, 128).transpose(0, 1, 3, 5, 4, 2)
    xt = np.ascontiguousarray(xt)

    # v-columns of the kv projection: Wkv reshaped (E, G, 2, D), kv index 1.
    wv = Wkv.reshape(E, G, 2, D)[:, :, 1, :].reshape(E, NV)  # (2048, 256)
    # e = ko*128 + p, col = nh*128 + n: (ko, p, nh, n) -> (nh, p, ko, n)
    wv_dev = np.ascontiguousarray(
        _to_bf16(wv).reshape(KO, 128, NH, 128).transpose(2, 1, 0, 3)
    )
    bv = bkv.reshape(G, 2, D)[:, 1, :].reshape(NV)  # (256,)
    bias_dev = np.ascontiguousarray(bv.reshape(NH, 128).T).astype(np.float32)
    return xt, wv_dev, bias_dev


def kernel(x, Wq, bq, Wkv, bkv):
    global LAST_RESULTS
    from concourse.bass_utils import run_bass_kernel_spmd

    nc = _get_nc()
    xt, wv_dev, bias_dev = _prep_inputs(x, Wkv, bkv)
    in_maps = [
        {"xt": xt[c], "wv": wv_dev, "bias": bias_dev} for c in range(NCORES)
    ]
    res = run_bass_kernel_spmd(nc, in_maps, core_ids=list(range(NCORES)))
    LAST_RESULTS = res
    # (NH, TB, 128n, TBLKt) -> (TB, t, NH, n) -> (TPC, NV)
    y = np.stack(
        [
            np.asarray(res.results[c]["out"])
            .astype(np.float32)
            .transpose(1, 3, 0, 2)
            .reshape(TPC, NV)
            for c in range(NCORES)
        ]
    ).reshape(TOK, NV)
    out = np.broadcast_to(
        y.reshape(TOK, G, 1, D), (TOK, G, HPG, D)
    ).reshape(B, L, E)
    return np.ascontiguousarray(out).astype(np.float32)


# revision 14
# speedup vs baseline: 1.0494x; 1.0494x over previous
"""TRN2 Bass kernel for nn_GQA_22436909154699 — optimized v5.

Reference math: softmax over a size-1 axis is identically 1.0, so
    out[b,l,g,h,:] = v[b,l,g,:]          (v = v-half of x @ Wkv + bkv)
The q projection (x @ Wq) never affects the output.  The kernel computes
    y = x @ Wv + bv                      (K=2048, N=256)
data-parallel over tokens across 8 NeuronCores (2048 tokens each).

Measured fixed framework cost is ~10us inside the counted window (~1us
preamble + ~8.6us end-of-NEFF semaphore sweep), so the optimizable span
is [first DMA .. last output landed]:
  - x streams as 16 x 512KB chunks on the Sync HWDGE ring (512KB keeps
    the SDMA engines near line rate; 256KB chunks measured only 78%).
  - weights (k-halves interleaved wv0a,wv1a,wv0b,wv1b) + bias ride the
    Scalar/ACT ring in parallel, so the first matmul only gates on
    ~0.5MB of weights + one x chunk.
  - per-chunk nh-interleave: each 4-ktile chunk feeds both column-half
    PSUM groups immediately; the PE trails the stream by one chunk and
    the post-stream tail is ~1.7us.
  - bf16 outputs, block outputs split across both rings (nh0 on Sync,
    nh1 on ACT) so the final adds + stores overlap.
  - 5 PE warm-ups on zeros cover the HAM un-throttle before real MMs.
"""

import numpy as np

# Problem constants (hardcoded; harness runs kernel.py standalone).
B, L, E = 4, 4096, 2048
G, HPG, D = 4, 8, 64
NV = G * D  # 256 v-columns
NH = NV // 128  # 2 column halves (PE stationary is 128 wide)
NCORES = 8
TOK = B * L  # 16384 tokens
TPC = TOK // NCORES  # 2048 tokens per core
TBLK = 512  # tokens per matmul rhs / PSUM group
TB = TPC // TBLK  # 4 token blocks per core
KO = E // 128  # 16 contraction tiles
XCH = 4  # x chunks per block (512 KB each)
KPC = KO // XCH  # k-tiles per chunk = 4
WH = 2  # weight k-halves per nh (256 KB each)

_CACHE: dict = {}
LAST_RESULTS = None


def _build(warmup: int):
    import concourse.bacc as bacc
    import concourse.mybir as mybir
    import concourse.tile as tile

    F32 = mybir.dt.float32
    BF16 = mybir.dt.bfloat16

    nc = bacc.Bacc(
        "TRN2", target_bir_lowering=False, debug=False, num_devices=NCORES
    )
    xt_d = nc.dram_tensor(
        "xt", [TB, XCH, 128, KPC, TBLK], BF16, kind="ExternalInput"
    )
    wv_d = nc.dram_tensor("wv", [NH, 128, KO, 128], BF16, kind="ExternalInput")
    bias_d = nc.dram_tensor("bias", [128, NH], F32, kind="ExternalInput")
    out_d = nc.dram_tensor("out", [NH, TB, 128, TBLK], BF16, kind="ExternalOutput")

    with tile.TileContext(nc) as tc:
        with (
            tc.tile_pool(name="const", bufs=1) as cpool,
            tc.tile_pool(name="xin", bufs=TB) as xpool,
            tc.tile_pool(name="obuf", bufs=4) as opool,
            tc.tile_pool(name="ps", bufs=8, space="PSUM") as ppool,
        ):
            # PE warm-up on zeros while the first DMAs land.  ~8 N=512 MMs
            # span the ~3.4us HAM busy-window, so the real MM stream starts
            # at K=8/8 (2.4 GHz) with no cold prefix.  GpSimd memset runs at
            # window start (DVE would add ~0.5us of latency).
            if warmup:
                zt = cpool.tile([128, TBLK], BF16)
                nc.gpsimd.memset(zt[:], 0.0)
                wps = ppool.tile([128, TBLK], F32, tag="ps")
                for _ in range(warmup):
                    nc.tensor.matmul(
                        wps[:], lhsT=zt[:, :128], rhs=zt[:], start=True, stop=True
                    )

            # Weights + bias on the ACT HWDGE ring.  One DMA per wv half:
            # Tile has only 8 global DMAHW completion lanes, and extra weight
            # DMAs here starve the x-chunk dispatches of lanes (measured: the
            # 4th x chunk's dispatch stalled ~3us behind weight completions).
            wvs = []
            for nh in range(NH):
                wvs.append(
                    cpool.tile(
                        [128, KO, 128], BF16, tag=f"wv{nh}", name=f"wv{nh}"
                    )
                )
                nc.scalar.dma_start(wvs[nh][:], wv_d[nh])
            bias_sb = cpool.tile([128, NH], F32)
            nc.scalar.dma_start(bias_sb[:], bias_d[:])

            # x stream: 16 x 512KB chunks, FIFO on the Sync HWDGE ring.
            xin = []
            for tb in range(TB):
                xt = xpool.tile([128, KO, TBLK], BF16, tag="xin")
                for c in range(XCH):
                    nc.sync.dma_start(
                        xt[:, c * KPC : (c + 1) * KPC, :], xt_d[tb, c]
                    )
                xin.append(xt)

            for tb in range(TB):
                pss = [
                    ppool.tile([128, TBLK], F32, tag="ps", name=f"ps{tb}_{i}")
                    for i in range(NH)
                ]
                for c in range(XCH):
                    for nh in range(NH):
                        for kk in range(KPC):
                            k = c * KPC + kk
                            nc.tensor.matmul(
                                pss[nh][:],
                                lhsT=wvs[nh][:, k, :],
                                rhs=xin[tb][:, k, :],
                                start=(k == 0),
                                stop=(k == KO - 1),
                            )
                for nh in range(NH):
                    ot = opool.tile([128, TBLK], BF16, tag="ot", name=f"ot{tb}_{nh}")
                    if nh == 0:
                        # DVE add + store on the Sync ring.
                        nc.vector.tensor_add(
                            ot[:],
                            pss[nh][:],
                            bias_sb[:, nh, None].to_broadcast([128, TBLK]),
                        )
                        nc.sync.dma_start(out_d[nh, tb], ot[:])
                    else:
                        # ACT copy-with-bias (faster than the DVE add) + store
                        # on the ACT ring — the nh1 drain is the critical tail.
                        nc.scalar.activation(
                            ot[:],
                            pss[nh][:],
                            mybir.ActivationFunctionType.Identity,
                            bias=bias_sb[:, nh, None],
                        )
                        nc.scalar.dma_start(out_d[nh, tb], ot[:])
    nc.compile()
    return nc


def _get_nc():
    warmup = 12
    key = ("nc8", warmup)
    if key not in _CACHE:
        _CACHE[key] = _build(warmup)
    return _CACHE[key]


def _to_bf16(a):
    import ml_dtypes

    return a.astype(ml_dtypes.bfloat16)


def _prep_inputs(x, Wkv, bkv):
    x = np.asarray(x, dtype=np.float32).reshape(TOK, E)
    Wkv = np.asarray(Wkv, dtype=np.float32)
    bkv = np.asarray(bkv, dtype=np.float32)

    xb = _to_bf16(x)
    # (core, tb, t, c, kk, p) -> (core, tb, c, p, kk, t)
    xt = xb.reshape(NCORES, TB, TBLK, XCH, KPC, 128).transpose(0, 1, 3, 5, 4, 2)
    xt = np.ascontiguousarray(xt)

    # v-columns of the kv projection: Wkv reshaped (E, G, 2, D), kv index 1.
    wv = Wkv.reshape(E, G, 2, D)[:, :, 1, :].reshape(E, NV)  # (2048, 256)
    # e = ko*128 + p, col = nh*128 + n: (ko, p, nh, n) -> (nh, p, ko, n)
    wv_dev = np.ascontiguousarray(
        _to_bf16(wv).reshape(KO, 128, NH, 128).transpose(2, 1, 0, 3)
    )
    bv = bkv.reshape(G, 2, D)[:, 1, :].reshape(NV)  # (256,)
    bias_dev = np.ascontiguousarray(bv.reshape(NH, 128).T).astype(np.float32)
    return xt, wv_dev, bias_dev


def kernel(x, Wq, bq, Wkv, bkv):
    global LAST_RESULTS
    from concourse.bass_utils import run_bass_kernel_spmd

    nc = _get_nc()
    xt, wv_dev, bias_dev = _prep_inputs(x, Wkv, bkv)
    in_maps = [
        {"xt": xt[c], "wv": wv_dev, "bias": bias_dev} for c in range(NCORES)
    ]
    res = run_bass_kernel_spmd(nc, in_maps, core_ids=list(range(NCORES)))
    LAST_RESULTS = res
    # (NH, TB, 128n, TBLKt) -> (TB, t, NH, n) -> (TPC, NV)
    y = np.stack(
        [
            np.asarray(res.results[c]["out"])
            .astype(np.float32)
            .transpose(1, 3, 0, 2)
            .reshape(TPC, NV)
            for c in range(NCORES)
        ]
    ).reshape(TOK, NV)
    out = np.broadcast_to(
        y.reshape(TOK, G, 1, D), (TOK, G, HPG, D)
    ).reshape(B, L, E)
    return np.ascontiguousarray(out).astype(np.float32)


# revision 19
# speedup vs baseline: 1.0771x; 1.0264x over previous
"""TRN2 Bass kernel for nn_GQA_22436909154699 — optimized v5.

Reference math: softmax over a size-1 axis is identically 1.0, so
    out[b,l,g,h,:] = v[b,l,g,:]          (v = v-half of x @ Wkv + bkv)
The q projection (x @ Wq) never affects the output.  The kernel computes
    y = x @ Wv + bv                      (K=2048, N=256)
data-parallel over tokens across 8 NeuronCores (2048 tokens each).

Measured fixed framework cost is ~10us inside the counted window (~1us
preamble + ~8.6us end-of-NEFF semaphore sweep), so the optimizable span
is [first DMA .. last output landed]:
  - x streams as 16 x 512KB chunks on the Sync HWDGE ring (512KB keeps
    the SDMA engines near line rate; 256KB chunks measured only 78%).
  - weights (k-halves interleaved wv0a,wv1a,wv0b,wv1b) + bias ride the
    Scalar/ACT ring in parallel, so the first matmul only gates on
    ~0.5MB of weights + one x chunk.
  - per-chunk nh-interleave: each 4-ktile chunk feeds both column-half
    PSUM groups immediately; the PE trails the stream by one chunk and
    the post-stream tail is ~1.7us.
  - bf16 outputs, block outputs split across both rings (nh0 on Sync,
    nh1 on ACT) so the final adds + stores overlap.
  - 5 PE warm-ups on zeros cover the HAM un-throttle before real MMs.
"""

import numpy as np

# Problem constants (hardcoded; harness runs kernel.py standalone).
B, L, E = 4, 4096, 2048
G, HPG, D = 4, 8, 64
NV = G * D  # 256 v-columns
NH = NV // 128  # 2 column halves (PE stationary is 128 wide)
NCORES = 8
TOK = B * L  # 16384 tokens
TPC = TOK // NCORES  # 2048 tokens per core
TBLK = 512  # tokens per matmul rhs / PSUM group
TB = TPC // TBLK  # 4 token blocks per core
KO = E // 128  # 16 contraction tiles
XCH = 4  # x chunks per block (512 KB each)
KPC = KO // XCH  # k-tiles per chunk = 4
WH = 2  # weight k-halves per nh (256 KB each)

_CACHE: dict = {}
LAST_RESULTS = None


def _build(warmup: int):
    import concourse.bacc as bacc
    import concourse.mybir as mybir
    import concourse.tile as tile

    F32 = mybir.dt.float32
    BF16 = mybir.dt.bfloat16

    nc = bacc.Bacc(
        "TRN2", target_bir_lowering=False, debug=False, num_devices=NCORES
    )
    xt_d = nc.dram_tensor(
        "xt", [TB, XCH, 128, KPC, TBLK], BF16, kind="ExternalInput"
    )
    wv_d = nc.dram_tensor(
        "wv", [XCH, 128, KPC, NH, 128], BF16, kind="ExternalInput"
    )
    bias_d = nc.dram_tensor("bias", [128, NH], F32, kind="ExternalInput")
    out_d = nc.dram_tensor("out", [NH, TB, 128, TBLK], BF16, kind="ExternalOutput")

    with tile.TileContext(nc) as tc:
        with (
            tc.tile_pool(name="const", bufs=1) as cpool,
            tc.tile_pool(name="xin", bufs=TB) as xpool,
            tc.tile_pool(name="obuf", bufs=4) as opool,
            tc.tile_pool(name="ps", bufs=8, space="PSUM") as ppool,
        ):
            # PE warm-up on zeros while the first DMAs land.  ~8 N=512 MMs
            # span the ~3.4us HAM busy-window, so the real MM stream starts
            # at K=8/8 (2.4 GHz) with no cold prefix.  GpSimd memset runs at
            # window start (DVE would add ~0.5us of latency).
            if warmup:
                zt = cpool.tile([128, TBLK], BF16)
                nc.gpsimd.memset(zt[:], 0.0)
                wps = ppool.tile([128, TBLK], F32, tag="ps")
                for _ in range(warmup):
                    nc.tensor.matmul(
                        wps[:], lhsT=zt[:, :128], rhs=zt[:], start=True, stop=True
                    )

            # Single weight tile [128, KO, NH, 128]; lhsT slices are
            # wv_all[:, k, nh, :].  Weight k-chunks interleave with the x
            # chunks on ONE FIFO ring, so the front-load before the first
            # real MM is just 256KB wv + 512KB x (measured: a 1MB weight
            # prefix delays the whole PE stream by ~3us).
            wv_all = cpool.tile([128, KO, NH, 128], BF16, name="wv_all")
            bias_sb = cpool.tile([128, NH], F32)
            nc.scalar.dma_start(bias_sb[:], bias_d[:])

            xin = [
                xpool.tile([128, KO, TBLK], BF16, tag="xin", name=f"xin{tb}")
                for tb in range(TB)
            ]
            # Sync-ring FIFO order: (wv c, x0 c) pairs, then the x stream.
            for c in range(XCH):
                nc.sync.dma_start(
                    wv_all[:, c * KPC : (c + 1) * KPC, :, :], wv_d[c]
                )
                nc.sync.dma_start(
                    xin[0][:, c * KPC : (c + 1) * KPC, :], xt_d[0, c]
                )
            for tb in range(1, TB):
                for c in range(XCH):
                    nc.sync.dma_start(
                        xin[tb][:, c * KPC : (c + 1) * KPC, :], xt_d[tb, c]
                    )

            for tb in range(TB):
                pss = [
                    ppool.tile([128, TBLK], F32, tag="ps", name=f"ps{tb}_{i}")
                    for i in range(NH)
                ]
                for c in range(XCH):
                    for nh in range(NH):
                        for kk in range(KPC):
                            k = c * KPC + kk
                            nc.tensor.matmul(
                                pss[nh][:],
                                lhsT=wv_all[:, k, nh, :],
                                rhs=xin[tb][:, k, :],
                                start=(k == 0),
                                stop=(k == KO - 1),
                            )
                for nh in range(NH):
                    ot = opool.tile([128, TBLK], BF16, tag="ot", name=f"ot{tb}_{nh}")
                    if nh == 0:
                        # DVE add + store on the Sync ring.
                        nc.vector.tensor_add(
                            ot[:],
                            pss[nh][:],
                            bias_sb[:, nh, None].to_broadcast([128, TBLK]),
                        )
                        nc.sync.dma_start(out_d[nh, tb], ot[:])
                    else:
                        # ACT copy-with-bias (faster than the DVE add) + store
                        # on the ACT ring — the nh1 drain is the critical tail.
                        nc.scalar.activation(
                            ot[:],
                            pss[nh][:],
                            mybir.ActivationFunctionType.Identity,
                            bias=bias_sb[:, nh, None],
                        )
                        nc.scalar.dma_start(out_d[nh, tb], ot[:])
    nc.compile()
    return nc


def _get_nc():
    warmup = 7
    key = ("nc9", warmup)
    if key not in _CACHE:
        _CACHE[key] = _build(warmup)
    return _CACHE[key]


def _to_bf16(a):
    import ml_dtypes

    return a.astype(ml_dtypes.bfloat16)


def _prep_inputs(x, Wkv, bkv):
    x = np.asarray(x, dtype=np.float32).reshape(TOK, E)
    Wkv = np.asarray(Wkv, dtype=np.float32)
    bkv = np.asarray(bkv, dtype=np.float32)

    xb = _to_bf16(x)
    # (core, tb, t, c, kk, p) -> (core, tb, c, p, kk, t)
    xt = xb.reshape(NCORES, TB, TBLK, XCH, KPC, 128).transpose(0, 1, 3, 5, 4, 2)
    xt = np.ascontiguousarray(xt)

    # v-columns of the kv projection: Wkv reshaped (E, G, 2, D), kv index 1.
    wv = Wkv.reshape(E, G, 2, D)[:, :, 1, :].reshape(E, NV)  # (2048, 256)
    # e = (c*KPC+kk)*128 + p, col = nh*128 + n:
    # (c, kk, p, nh, n) -> (c, p, kk, nh, n)
    wv_dev = np.ascontiguousarray(
        _to_bf16(wv).reshape(XCH, KPC, 128, NH, 128).transpose(0, 2, 1, 3, 4)
    )
    bv = bkv.reshape(G, 2, D)[:, 1, :].reshape(NV)  # (256,)
    bias_dev = np.ascontiguousarray(bv.reshape(NH, 128).T).astype(np.float32)
    return xt, wv_dev, bias_dev


def kernel(x, Wq, bq, Wkv, bkv):
    global LAST_RESULTS
    from concourse.bass_utils import run_bass_kernel_spmd

    nc = _get_nc()
    xt, wv_dev, bias_dev = _prep_inputs(x, Wkv, bkv)
    in_maps = [
        {"xt": xt[c], "wv": wv_dev, "bias": bias_dev} for c in range(NCORES)
    ]
    res = run_bass_kernel_spmd(nc, in_maps, core_ids=list(range(NCORES)))
    LAST_RESULTS = res
    # (NH, TB, 128n, TBLKt) -> (TB, t, NH, n) -> (TPC, NV)
    y = np.stack(
        [
            np.asarray(res.results[c]["out"])
            .astype(np.float32)
            .transpose(1, 3, 0, 2)
            .reshape(TPC, NV)
            for c in range(NCORES)
        ]
    ).reshape(TOK, NV)
    out = np.broadcast_to(
        y.reshape(TOK, G, 1, D), (TOK, G, HPG, D)
    ).reshape(B, L, E)
    return np.ascontiguousarray(out).astype(np.float32)
